# revision 22
# baseline (speedup 1.0000x reference)
"""ROI crop-and-pool (bilinear grid sample + 2x2 max pool) on 8 NeuronCores.

Strategy: data-parallel over the 512 ROIs (64 per core). Every pooled output
"slot" (ROI x 7x7 position) needs 16 feature-map points: 2x2 pool members x 4
bilinear corners. The host packs the feature map as a "quad table"
featQ[y*W+x] = [f(y,x), f(y,x+1), f(y+1,x), f(y+1,x+1)] (edge-clamped), so one
indirect DMA row fetch (per-partition offset) brings all 4 corners of one
sample point.

Default design "g8i6": the quad table is stored in fp8-e3m4 (halves gather
bytes; 4-bit mantissa keeps the deterministic L2 rel err at 1.29e-2, under
the 2e-2 gate — e4m3 measures 2.56e-2 and fails). Per chunk of 128 slots,
4 indirect DMAs (one per 2x2 pool member, 128 x 2KB rows each) fill an fp8
tile; the 16 weighted corner adds run on the TensorEngine as diag-matmuls
(DVE builds diag(w) fp16 from an identity via tensor_scalar at 4x rate; PE
contracts diag @ corner-block into fp32 PSUM, mixed fp16 x fp8 operands).
Max-pool obeys the one-PSUM-operand rule: ScalarE evacuates samples 1/3 to
SBUF fp16, DVE does the three maxes, one DMA stores each chunk. A 6-deep
gather pipeline (gpool bufs) keeps the indirect-DMA queue full — measured
HW is descriptor-rate-bound (~13 ns/descriptor), so gather depth, not
bandwidth, dominates. Measured ~131 us vs the 219 us fp16/DVE baseline.

Older designs kept for comparison: q16 (fp16 + DVE/ScalarE weighted sums),
q16pe, g8 (dma_gather variant — slower: Q7 SWDGE gather underperforms
SWDGE indirect DMA per descriptor), x32 (fp32 fallback).
"""

import numpy as np

POOL = 7
PRE = POOL * 2          # 14
STRIDE = 16.0
C, H, W = 512, 50, 75
N = 512
NCORES = 8
N_LOC = N // NCORES     # 64 ROIs per core
SLOTS = N_LOC * POOL * POOL          # 3136 pooled outputs per core
CHUNKS = (SLOTS + 127) // 128        # 25
SLOT_PAD = CHUNKS * 128              # 3200
NW = 16                              # weights per slot

DESIGN = "g8i6"  # fp8e3 quads via indirect DMA + PE diag-matmul + PSUM pooling

_CACHE = {}


def _axis_corners(s, t, size):
    """Sample positions v -> floor corner v0 and corner weights w0/w1 (fp32)."""
    f32 = np.float32
    base = np.linspace(-1.0, 1.0, PRE, dtype=f32)
    g = s[:, None] * base[None, :] + t[:, None]          # [N, 14]
    v = (g + f32(1.0)) * f32(0.5) * f32(size - 1)
    v0 = np.floor(v)
    w1 = v - v0
    w0 = f32(1.0) - w1
    return v0, w0, w1


def _roi_params(rois):
    f32 = np.float32
    r = rois.astype(f32)
    x1 = r[:, 1] / f32(STRIDE)
    y1 = r[:, 2] / f32(STRIDE)
    x2 = r[:, 3] / f32(STRIDE)
    y2 = r[:, 4] / f32(STRIDE)
    sx = (x2 - x1) / f32(W - 1)
    tx = (x1 + x2 - W + 1) / f32(W - 1)
    sy = (y2 - y1) / f32(H - 1)
    ty = (y1 + y2 - H + 1) / f32(H - 1)
    return sx, tx, sy, ty


def _clip_remap(v0, w0, w1, size, start_max):
    """Clip unit start to [0, start_max]; distribute corner weights onto the
    unit-local positions d = (v0 + c) - start, dropping invalid corners."""
    f32 = np.float32
    start = np.clip(v0, 0, start_max).astype(np.int32)
    wd = np.zeros(v0.shape + (2,), f32)
    for c in range(2):
        vc = v0 + f32(c)
        valid = (vc >= 0) & (vc <= size - 1)
        wc = (w0 if c == 0 else w1) * valid.astype(f32)
        d = vc.astype(np.int64) - start
        for dd in range(2):
            wd[..., dd] += np.where((d == dd) & valid, wc, 0.0).astype(f32)
    return start, wd


def _host_prep_q16(bottom, rois):
    """Quad-table design: featQ fp16 [H*W, 4C]; 4 gathers per chunk."""
    f = bottom[0].transpose(1, 2, 0)                   # [H, W, C] fp32
    fq = np.empty((H, W, 4, C), np.float16)
    fx = f[:, list(range(1, W)) + [W - 1], :]          # x+1 clamped
    fy = f[list(range(1, H)) + [H - 1], :, :]          # y+1 clamped
    fxy = fy[:, list(range(1, W)) + [W - 1], :]
    fq[:, :, 0] = f
    fq[:, :, 1] = fx
    fq[:, :, 2] = fy
    fq[:, :, 3] = fxy
    featQ = np.ascontiguousarray(fq.reshape(H * W, 4 * C))

    sx, tx, sy, ty = _roi_params(rois)
    y0, wy0, wy1 = _axis_corners(sy, ty, H)
    x0, wx0, wx1 = _axis_corners(sx, tx, W)
    ys, wyd = _clip_remap(y0, wy0, wy1, H, H - 1)      # [N,14], [N,14,2]
    xs, wxd = _clip_remap(x0, wx0, wx1, W, W - 1)

    in_maps = []
    for k in range(NCORES):
        sl = slice(k * N_LOC, (k + 1) * N_LOC)
        ys_v = ys[sl].reshape(N_LOC, POOL, 2)          # [n, I, a]
        wy_v = wyd[sl].reshape(N_LOC, POOL, 2, 2)      # [n, I, a, dy]
        xs_v = xs[sl].reshape(N_LOC, POOL, 2)          # [n, J, b]
        wx_v = wxd[sl].reshape(N_LOC, POOL, 2, 2)      # [n, J, b, dx]

        # unit (a, b): row = ys*W + xs -> [n, I, J, a, b]
        idx_all = (
            ys_v[:, :, None, :, None] * W + xs_v[:, None, :, None, :]
        )
        # weight (a, b, dy, dx) -> [n, I, J, a, b, dy, dx]
        w_all = (
            wy_v[:, :, None, :, None, :, None]
            * wx_v[:, None, :, None, :, None, :]
        )
        idx_flat = idx_all.reshape(SLOTS, 4)
        w_flat = w_all.reshape(SLOTS, NW).astype(np.float32)
        idx_pad = np.zeros((SLOT_PAD, 4), np.int32)
        w_pad = np.zeros((SLOT_PAD, NW), np.float32)
        idx_pad[:SLOTS] = idx_flat
        w_pad[:SLOTS] = w_flat

        idx_dev = (
            idx_pad.reshape(CHUNKS, 128, 4)
            .transpose(1, 0, 2).reshape(128, CHUNKS * 4).copy()
        )
        w_dev = (
            w_pad.reshape(CHUNKS, 128, NW)
            .transpose(1, 0, 2).reshape(128, CHUNKS * NW).copy()
        )
        in_maps.append({"featQ": featQ, "idxs": idx_dev, "wts": w_dev,
                        "ident": np.eye(128, dtype=np.float16)})
    return in_maps


def _build_q16(repeat=1):
    import concourse.bacc as bacc
    import concourse.bass as bass
    import concourse.tile as tile
    from concourse import mybir
    from concourse.bass_interp import get_hw_module

    f16 = mybir.dt.float16
    nc = bacc.Bacc("TRN2", target_bir_lowering=False, debug=False,
                   num_devices=NCORES)
    featQ = nc.dram_tensor("featQ", (H * W, 4 * C), f16, kind="ExternalInput")
    idx_d = nc.dram_tensor("idxs", (128, CHUNKS * 4), mybir.dt.int32,
                           kind="ExternalInput")
    wts_d = nc.dram_tensor("wts", (128, CHUNKS * NW), mybir.dt.float32,
                           kind="ExternalInput")
    out_d = nc.dram_tensor("out", (CHUNKS, 128, C), f16,
                           kind="ExternalOutput")

    U = 4 * C  # elements per gathered unit (4 corners)

    with tile.TileContext(nc) as tc:
        with tc.tile_pool(name="cpool", bufs=1) as cpool, \
             tc.tile_pool(name="gpool", bufs=8) as gpool, \
             tc.tile_pool(name="tpool", bufs=6) as tpool, \
             tc.tile_pool(name="opool", bufs=3) as opool:
            idx_sb = cpool.tile([128, CHUNKS * 4], mybir.dt.int32, tag="idx")
            wts_sb = cpool.tile([128, CHUNKS * NW], mybir.dt.float32,
                                tag="wts")
            nc.sync.dma_start(out=idx_sb[:], in_=idx_d[:])
            nc.sync.dma_start(out=wts_sb[:], in_=wts_d[:])

            def body():
                for ch in range(CHUNKS):
                    g = gpool.tile([128, 4 * U], f16, tag="g")
                    for m in range(4):
                        nc.gpsimd.indirect_dma_start(
                            out=g[:, m * U:(m + 1) * U],
                            out_offset=None,
                            in_=featQ[:],
                            in_offset=bass.IndirectOffsetOnAxis(
                                ap=idx_sb[:, ch * 4 + m: ch * 4 + m + 1],
                                axis=0,
                            ),
                        )
                    accs = []
                    for m in range(4):
                        acc = tpool.tile([128, C], f16, tag=f"acc{m}")
                        s1 = tpool.tile([128, C], f16, tag="s1")
                        s2 = tpool.tile([128, C], f16, tag="s2")
                        s3 = tpool.tile([128, C], f16, tag="s3")
                        for q, t in enumerate((acc, s1, s2, s3)):
                            wcol = ch * NW + m * 4 + q
                            src = g[:, m * U + q * C: m * U + (q + 1) * C]
                            wap = wts_sb[:, wcol:wcol + 1]
                            if q < 2:
                                nc.vector.tensor_scalar_mul(t[:], src, wap)
                            else:
                                nc.scalar.mul(t[:], src, wap)
                        nc.vector.tensor_add(acc[:], acc[:], s1[:])
                        nc.vector.tensor_add(s2[:], s2[:], s3[:])
                        nc.vector.tensor_add(acc[:], acc[:], s2[:])
                        accs.append(acc)
                    nc.vector.tensor_max(accs[0][:], accs[0][:], accs[1][:])
                    nc.vector.tensor_max(accs[2][:], accs[2][:], accs[3][:])
                    ot = opool.tile([128, C], f16, tag="o")
                    nc.vector.tensor_max(ot[:], accs[0][:], accs[2][:])
                    nc.sync.dma_start(out=out_d[ch], in_=ot[:])

            if repeat > 1:
                with tc.For_i(0, repeat, 1):
                    body()
            else:
                body()

    nc.compile()
    nc.m = get_hw_module(nc.m)
    return nc


def _build_q16pe(repeat=1):
    """Like q16, but the 16 weighted-corner multiplies + 12 adds run on the
    TensorEngine as diagonal-matrix matmuls accumulating in PSUM (fp32).
    Each diag is built by one cheap DVE tensor_scalar (identity mask x w).
    ScalarE evacuates PSUM -> SBUF; VectorE does the 3 max-pool ops."""
    import concourse.bacc as bacc
    import concourse.bass as bass
    import concourse.tile as tile
    from concourse import mybir
    from concourse.bass_interp import get_hw_module

    f16 = mybir.dt.float16
    f32 = mybir.dt.float32
    nc = bacc.Bacc("TRN2", target_bir_lowering=False, debug=False,
                   num_devices=NCORES)
    featQ = nc.dram_tensor("featQ", (H * W, 4 * C), f16, kind="ExternalInput")
    idx_d = nc.dram_tensor("idxs", (128, CHUNKS * 4), mybir.dt.int32,
                           kind="ExternalInput")
    wts_d = nc.dram_tensor("wts", (128, CHUNKS * NW), f32,
                           kind="ExternalInput")
    id_d = nc.dram_tensor("ident", (128, 128), f16, kind="ExternalInput")
    out_d = nc.dram_tensor("out", (CHUNKS, 128, C), f16,
                           kind="ExternalOutput")

    U = 4 * C

    with tile.TileContext(nc) as tc:
        with tc.tile_pool(name="cpool", bufs=1) as cpool, \
             tc.tile_pool(name="gpool", bufs=8) as gpool, \
             tc.tile_pool(name="dpool", bufs=8) as dpool, \
             tc.tile_pool(name="tpool", bufs=4) as tpool, \
             tc.tile_pool(name="ppool", bufs=2, space="PSUM") as ppool, \
             tc.tile_pool(name="opool", bufs=3) as opool:
            idx_sb = cpool.tile([128, CHUNKS * 4], mybir.dt.int32, tag="idx")
            wts_sb = cpool.tile([128, CHUNKS * NW], f32, tag="wts")
            id_sb = cpool.tile([128, 128], f16, tag="ident")
            nc.sync.dma_start(out=idx_sb[:], in_=idx_d[:])
            nc.sync.dma_start(out=wts_sb[:], in_=wts_d[:])
            nc.sync.dma_start(out=id_sb[:], in_=id_d[:])

            def body():
                for ch in range(CHUNKS):
                    g = gpool.tile([128, 4 * U], f16, tag="g")
                    for m in range(4):
                        nc.gpsimd.indirect_dma_start(
                            out=g[:, m * U:(m + 1) * U],
                            out_offset=None,
                            in_=featQ[:],
                            in_offset=bass.IndirectOffsetOnAxis(
                                ap=idx_sb[:, ch * 4 + m: ch * 4 + m + 1],
                                axis=0,
                            ),
                        )
                    sms = []
                    for m in range(4):
                        pacc = ppool.tile([128, C], f32, tag=f"p{m}",
                                          space="PSUM")
                        for q in range(4):
                            wcol = ch * NW + m * 4 + q
                            dg = dpool.tile([128, 128], f16, tag="d")
                            nc.vector.tensor_scalar_mul(
                                dg[:], id_sb[:], wts_sb[:, wcol:wcol + 1]
                            )
                            nc.tensor.matmul(
                                pacc[:],
                                lhsT=dg[:],
                                rhs=g[:, m * U + q * C: m * U + (q + 1) * C],
                                start=(q == 0),
                                stop=(q == 3),
                            )
                        sm = tpool.tile([128, C], f16, tag=f"s{m}")
                        nc.scalar.copy(sm[:], pacc[:])
                        sms.append(sm)
                    nc.vector.tensor_max(sms[0][:], sms[0][:], sms[1][:])
                    nc.vector.tensor_max(sms[2][:], sms[2][:], sms[3][:])
                    ot = opool.tile([128, C], f16, tag="o")
                    nc.vector.tensor_max(ot[:], sms[0][:], sms[2][:])
                    nc.sync.dma_start(out=out_d[ch], in_=ot[:])

            if repeat > 1:
                with tc.For_i(0, repeat, 1):
                    body()
            else:
                body()

    nc.compile()
    nc.m = get_hw_module(nc.m)
    return nc


def _host_prep_g8(bottom, rois):
    """fp8-e3m4 quad table + batched dma_gather + PE weighted sum.

    featQ8 [H*W, 4C] fp8e3 (quantization: L2 rel err ~1.3e-2, under the 2e-2
    gate). Per chunk of 128 slots one dma_gather fetches 512 quad rows
    (4 sample points per slot); indices int16 in the SWDGE wrapped layout
    (idx i at partition i%16, column i//16, replicated per 16-partition
    group). Weights fp32 per-partition (consumed as tensor_scalar scalars).
    """
    import ml_dtypes

    f = bottom[0].transpose(1, 2, 0)                   # [H, W, C] fp32
    fq = np.empty((H, W, 4, C), np.float32)
    fx = f[:, list(range(1, W)) + [W - 1], :]
    fy = f[list(range(1, H)) + [H - 1], :, :]
    fxy = fy[:, list(range(1, W)) + [W - 1], :]
    fq[:, :, 0] = f
    fq[:, :, 1] = fx
    fq[:, :, 2] = fy
    fq[:, :, 3] = fxy
    featQ8 = np.ascontiguousarray(
        fq.reshape(H * W, 4 * C).astype(ml_dtypes.float8_e3m4)
    )

    sx, tx, sy, ty = _roi_params(rois)
    y0, wy0, wy1 = _axis_corners(sy, ty, H)
    x0, wx0, wx1 = _axis_corners(sx, tx, W)
    ys, wyd = _clip_remap(y0, wy0, wy1, H, H - 1)
    xs, wxd = _clip_remap(x0, wx0, wx1, W, W - 1)

    ident = np.eye(128, dtype=np.float16)
    in_maps = []
    for k in range(NCORES):
        sl = slice(k * N_LOC, (k + 1) * N_LOC)
        ys_v = ys[sl].reshape(N_LOC, POOL, 2)
        wy_v = wyd[sl].reshape(N_LOC, POOL, 2, 2)
        xs_v = xs[sl].reshape(N_LOC, POOL, 2)
        wx_v = wxd[sl].reshape(N_LOC, POOL, 2, 2)

        idx_all = (
            ys_v[:, :, None, :, None] * W + xs_v[:, None, :, None, :]
        )
        w_all = (
            wy_v[:, :, None, :, None, :, None]
            * wx_v[:, None, :, None, :, None, :]
        )
        idx_flat = idx_all.reshape(SLOTS, 4)
        w_flat = w_all.reshape(SLOTS, NW).astype(np.float32)
        # sort slots by their first sample's cell so each gather's 128
        # descriptors hit ascending HBM addresses (better DRAM locality;
        # ~6% on the gather). _assemble inverts via _G8_PERMS.
        order = np.argsort(idx_flat[:, 0], kind="stable")
        idx_flat = idx_flat[order]
        w_flat = w_flat[order]
        _G8_PERMS[k] = order
        idx_pad = np.zeros((SLOT_PAD, 4), np.int64)
        w_pad = np.zeros((SLOT_PAD, NW), np.float32)
        idx_pad[:SLOTS] = idx_flat
        w_pad[:SLOTS] = w_flat

        # dma_gather ordering: gathered row i lands at out[i % 128, i // 128];
        # we want out[p, m] = quad row of (slot p, sample m) -> flat[i] with
        # i = m*128 + p. SWDGE wrapped layout: sb[i%16, i//16], replicated
        # across the eight 16-partition groups.
        idx_cols = []
        for ch in range(CHUNKS):
            flat = idx_pad[ch * 128:(ch + 1) * 128, :].T.reshape(512)
            blk = flat.reshape(32, 16).T            # [16, 32]
            idx_cols.append(np.tile(blk, (8, 1)))   # [128, 32]
        idx_dev = np.concatenate(idx_cols, axis=1).astype(np.int16)

        idx32_dev = (
            idx_pad.reshape(CHUNKS, 128, 4)
            .transpose(1, 0, 2).reshape(128, CHUNKS * 4).astype(np.int32)
        )
        w_dev = (
            w_pad.reshape(CHUNKS, 128, NW)
            .transpose(1, 0, 2).reshape(128, CHUNKS * NW).copy()
        )
        in_maps.append({"featQ8": featQ8, "idxs": idx_dev,
                        "idxs32": idx32_dev, "wts": w_dev, "ident": ident})
    return in_maps


def _build_g8(repeat=1, gather="dg", gbufs=3):
    """fp8e3 quad gather + PE diag-matmul weighted sum (lhsT fp16 diag built
    on DVE, rhs fp8e3, PSUM fp32 accumulate) + PSUM max-pool (Act evacuates
    samples 1/3, DVE maxes). gather: "dg" = one dma_gather per chunk,
    "idma" = four indirect DMAs per chunk (measured faster per descriptor)."""
    import concourse.bacc as bacc
    import concourse.bass as bass
    import concourse.tile as tile
    from concourse import mybir
    from concourse.bass_interp import get_hw_module

    f16 = mybir.dt.float16
    f32 = mybir.dt.float32
    f8 = mybir.dt.float8e3
    i16 = mybir.dt.int16
    nc = bacc.Bacc("TRN2", target_bir_lowering=False, debug=False,
                   num_devices=NCORES)
    featQ8 = nc.dram_tensor("featQ8", (H * W, 4 * C), f8, kind="ExternalInput")
    idx_d = nc.dram_tensor("idxs", (128, CHUNKS * 32), i16,
                           kind="ExternalInput")
    idx32_d = nc.dram_tensor("idxs32", (128, CHUNKS * 4), mybir.dt.int32,
                             kind="ExternalInput")
    wts_d = nc.dram_tensor("wts", (128, CHUNKS * NW), f32,
                           kind="ExternalInput")
    id_d = nc.dram_tensor("ident", (128, 128), f16, kind="ExternalInput")
    out_d = nc.dram_tensor("out", (CHUNKS, 128, C), f16,
                           kind="ExternalOutput")

    U = 4 * C   # fp8 elements per quad row (2048 bytes)

    with tile.TileContext(nc) as tc:
        with tc.tile_pool(name="cpool", bufs=1) as cpool, \
             tc.tile_pool(name="gpool", bufs=gbufs) as gpool, \
             tc.tile_pool(name="dpool", bufs=3) as dpool, \
             tc.tile_pool(name="ppool", bufs=2, space="PSUM") as ppool, \
             tc.tile_pool(name="spool", bufs=3) as spool, \
             tc.tile_pool(name="opool", bufs=3) as opool:
            idx_sb = cpool.tile([128, CHUNKS * 32], i16, tag="idx")
            idx32_sb = cpool.tile([128, CHUNKS * 4], mybir.dt.int32,
                                  tag="idx32")
            wts_sb = cpool.tile([128, CHUNKS * NW], f32, tag="wts")
            id_sb = cpool.tile([128, 128], f16, tag="ident")
            if gather == "dg":
                nc.sync.dma_start(out=idx_sb[:], in_=idx_d[:])
            else:
                nc.sync.dma_start(out=idx32_sb[:], in_=idx32_d[:])
            nc.sync.dma_start(out=wts_sb[:], in_=wts_d[:])
            nc.sync.dma_start(out=id_sb[:], in_=id_d[:])

            def body():
                for ch in range(CHUNKS):
                    g = gpool.tile([128, 4, U], f8, tag="g")
                    if gather == "dg":
                        nc.gpsimd.dma_gather(
                            g[:], featQ8[:], idx_sb[:, ch * 32:(ch + 1) * 32],
                            512, 512, U,
                        )
                    else:
                        for m in range(4):
                            nc.gpsimd.indirect_dma_start(
                                out=g[:, m, :],
                                out_offset=None,
                                in_=featQ8[:],
                                in_offset=bass.IndirectOffsetOnAxis(
                                    ap=idx32_sb[:, ch * 4 + m: ch * 4 + m + 1],
                                    axis=0,
                                ),
                            )
                    paccs = []
                    for m in range(4):
                        pm = ppool.tile([128, C], f32, tag=f"p{m}",
                                        space="PSUM")
                        for q in range(4):
                            col = ch * NW + m * 4 + q
                            dg = dpool.tile([128, 128], f16, tag=f"dg{m}{q}")
                            nc.vector.tensor_scalar_mul(
                                dg[:], id_sb[:], wts_sb[:, col:col + 1]
                            )
                            nc.tensor.matmul(
                                pm[:],
                                lhsT=dg[:],
                                rhs=g[:, m, q * C:(q + 1) * C],
                                start=(q == 0),
                                stop=(q == 3),
                            )
                        paccs.append(pm)
                    # HW rule: a TensorTensor may read at most one PSUM
                    # operand. Act evacuates samples 1/3; DVE maxes each
                    # against the PSUM-resident samples 0/2; gpsimd folds.
                    s1 = spool.tile([128, C], f16, tag="s1")
                    nc.scalar.copy(s1[:], paccs[1][:])
                    s3 = spool.tile([128, C], f16, tag="s3")
                    nc.scalar.copy(s3[:], paccs[3][:])
                    m01 = spool.tile([128, C], f16, tag="m01")
                    nc.vector.tensor_max(m01[:], paccs[0][:], s1[:])
                    m23 = spool.tile([128, C], f16, tag="m23")
                    nc.vector.tensor_max(m23[:], paccs[2][:], s3[:])
                    ot = opool.tile([128, C], f16, tag="o")
                    nc.vector.tensor_max(ot[:], m01[:], m23[:])
                    nc.sync.dma_start(out=out_d[ch], in_=ot[:])

            if repeat > 1:
                with tc.For_i(0, repeat, 1):
                    body()
            else:
                body()

    nc.compile()
    nc.m = get_hw_module(nc.m)
    return nc


def _host_prep_x32(bottom, rois):
    """fp32 fallback: featT [H*W, C] fp32; 8 x-pair gathers per chunk."""
    featT = np.ascontiguousarray(
        bottom[0].transpose(1, 2, 0).reshape(H * W, C), dtype=np.float32
    )
    sx, tx, sy, ty = _roi_params(rois)
    f32 = np.float32
    y0, wy0, wy1 = _axis_corners(sy, ty, H)
    yi = np.zeros(y0.shape + (2,), np.int32)
    wy = np.zeros(y0.shape + (2,), f32)
    for c in range(2):
        yc = y0 + f32(c)
        valid = (yc >= 0) & (yc <= H - 1)
        yi[..., c] = np.clip(yc, 0, H - 1).astype(np.int32)
        wy[..., c] = (wy0 if c == 0 else wy1) * valid.astype(f32)
    x0, wx0, wx1 = _axis_corners(sx, tx, W)
    xs, wxh = _clip_remap(x0, wx0, wx1, W, W - 2)

    in_maps = []
    for k in range(NCORES):
        sl = slice(k * N_LOC, (k + 1) * N_LOC)
        yi_v = yi[sl].reshape(N_LOC, POOL, 2, 2)     # [n, I, a, cy]
        wy_v = wy[sl].reshape(N_LOC, POOL, 2, 2)
        xs_v = xs[sl].reshape(N_LOC, POOL, 2)        # [n, J, b]
        wx_v = wxh[sl].reshape(N_LOC, POOL, 2, 2)    # [n, J, b, h]

        idx_all = (
            yi_v[:, :, None, :, None, :] * W
            + xs_v[:, None, :, None, :, None]
        )                                            # [n, I, J, a, b, cy]
        w_all = (
            wy_v[:, :, None, :, None, :, None]
            * wx_v[:, None, :, None, :, None, :]
        )                                            # [n, I, J, a, b, cy, h]
        idx_flat = idx_all.reshape(SLOTS, 8)
        w_flat = w_all.reshape(SLOTS, NW).astype(np.float32)
        idx_pad = np.zeros((SLOT_PAD, 8), np.int32)
        w_pad = np.zeros((SLOT_PAD, NW), np.float32)
        idx_pad[:SLOTS] = idx_flat
        w_pad[:SLOTS] = w_flat

        idx_dev = (
            idx_pad.reshape(CHUNKS, 128, 8)
            .transpose(1, 0, 2).reshape(128, CHUNKS * 8).copy()
        )
        w_dev = (
            w_pad.reshape(CHUNKS, 128, NW)
            .transpose(1, 0, 2).reshape(128, CHUNKS * NW).copy()
        )
        in_maps.append({"featT": featT, "idxs": idx_dev, "wts": w_dev})
    return in_maps


def _build_x32(repeat=1):
    import concourse.bacc as bacc
    import concourse.bass as bass
    import concourse.tile as tile
    from concourse import mybir
    from concourse.bass_interp import get_hw_module

    f32 = mybir.dt.float32
    nc = bacc.Bacc("TRN2", target_bir_lowering=False, debug=False,
                   num_devices=NCORES)
    featT = nc.dram_tensor("featT", (H * W, C), f32, kind="ExternalInput")
    idx_d = nc.dram_tensor("idxs", (128, CHUNKS * 8), mybir.dt.int32,
                           kind="ExternalInput")
    wts_d = nc.dram_tensor("wts", (128, CHUNKS * NW), f32,
                           kind="ExternalInput")
    out_d = nc.dram_tensor("out", (CHUNKS, 128, C), f32,
                           kind="ExternalOutput")

    U = 2 * C

    with tile.TileContext(nc) as tc:
        with tc.tile_pool(name="cpool", bufs=1) as cpool, \
             tc.tile_pool(name="gpool", bufs=3) as gpool, \
             tc.tile_pool(name="tpool", bufs=3) as tpool, \
             tc.tile_pool(name="opool", bufs=3) as opool:
            idx_sb = cpool.tile([128, CHUNKS * 8], mybir.dt.int32, tag="idx")
            wts_sb = cpool.tile([128, CHUNKS * NW], f32, tag="wts")
            nc.sync.dma_start(out=idx_sb[:], in_=idx_d[:])
            nc.sync.dma_start(out=wts_sb[:], in_=wts_d[:])

            def body():
                for ch in range(CHUNKS):
                    g = gpool.tile([128, 8 * U], f32, tag="g")
                    for u in range(8):
                        nc.gpsimd.indirect_dma_start(
                            out=g[:, u * U:(u + 1) * U],
                            out_offset=None,
                            in_=featT[:],
                            in_offset=bass.IndirectOffsetOnAxis(
                                ap=idx_sb[:, ch * 8 + u: ch * 8 + u + 1],
                                axis=0,
                            ),
                        )
                    accs = []
                    for m in range(4):
                        acc = tpool.tile([128, C], f32, tag=f"acc{m}")
                        s1 = tpool.tile([128, C], f32, tag="s1")
                        s2 = tpool.tile([128, C], f32, tag="s2")
                        s3 = tpool.tile([128, C], f32, tag="s3")
                        for q, t in enumerate((acc, s1, s2, s3)):
                            cy, hh = q // 2, q % 2
                            u = 2 * m + cy
                            wcol = ch * NW + u * 2 + hh
                            nc.scalar.mul(
                                t[:],
                                g[:, u * U + hh * C: u * U + (hh + 1) * C],
                                wts_sb[:, wcol:wcol + 1],
                            )
                        nc.vector.tensor_add(acc[:], acc[:], s1[:])
                        nc.vector.tensor_add(s2[:], s2[:], s3[:])
                        nc.vector.tensor_add(acc[:], acc[:], s2[:])
                        accs.append(acc)
                    nc.vector.tensor_max(accs[0][:], accs[0][:], accs[1][:])
                    nc.vector.tensor_max(accs[2][:], accs[2][:], accs[3][:])
                    ot = opool.tile([128, C], f32, tag="o")
                    nc.vector.tensor_max(ot[:], accs[0][:], accs[2][:])
                    nc.sync.dma_start(out=out_d[ch], in_=ot[:])

            if repeat > 1:
                with tc.For_i(0, repeat, 1):
                    body()
            else:
                body()

    nc.compile()
    nc.m = get_hw_module(nc.m)
    return nc


_DESIGNS = {
    "q16": (_host_prep_q16, _build_q16),
    "q16pe": (_host_prep_q16, _build_q16pe),
    "g8": (_host_prep_g8, _build_g8),
    "g8i": (_host_prep_g8,
            lambda repeat=1: _build_g8(repeat, gather="idma")),
    "g8i6": (_host_prep_g8,
             lambda repeat=1: _build_g8(repeat, gather="idma", gbufs=6)),
    "g8i10": (_host_prep_g8,
              lambda repeat=1: _build_g8(repeat, gather="idma", gbufs=10)),
    "x32": (_host_prep_x32, _build_x32),
}


def _get_program(design, repeat=1):
    key = (design, repeat)
    if key not in _CACHE:
        _CACHE[key] = _DESIGNS[design][1](repeat)
    return _CACHE[key]


_G8_PERMS = {}   # per-core slot sort order (g8 designs); see _host_prep_g8


def _assemble(outs, perms=None):
    """outs: list of per-core [CHUNKS, 128, C] arrays -> [N, C, 7, 7]."""
    full = np.empty((N, C, POOL, POOL), np.float32)
    for k, o in enumerate(outs):
        flat = np.asarray(o, np.float32).reshape(SLOT_PAD, C)[:SLOTS]
        if perms is not None:
            unperm = np.empty_like(flat)
            unperm[perms[k]] = flat
            flat = unperm
        full[k * N_LOC:(k + 1) * N_LOC] = (
            flat.reshape(N_LOC, POOL * POOL, C)
            .transpose(0, 2, 1)
            .reshape(N_LOC, C, POOL, POOL)
        )
    return full


def run_hw(bottom, rois, design=DESIGN, repeat=1, trace=False):
    from concourse import bass_utils

    in_maps = _DESIGNS[design][0](np.asarray(bottom), np.asarray(rois))
    nc = _get_program(design, repeat)
    res = bass_utils.run_bass_kernel_spmd(
        nc, in_maps, core_ids=list(range(NCORES)), trace=trace
    )
    perms = _G8_PERMS if design.startswith("g8") else None
    out = _assemble([r["out"] for r in res.results], perms=perms)
    return out, res


def kernel(bottom, rois):
    out, _ = run_hw(bottom, rois)
    return out



# revision 23
# speedup vs baseline: 1.0636x; 1.0636x over previous
"""ROI crop-and-pool (bilinear grid sample + 2x2 max pool) on 8 NeuronCores.

Strategy: data-parallel over the 512 ROIs (64 per core). Every pooled output
"slot" (ROI x 7x7 position) needs 16 feature-map points: 2x2 pool members x 4
bilinear corners. The host packs the feature map as a "quad table"
featQ[y*W+x] = [f(y,x), f(y,x+1), f(y+1,x), f(y+1,x+1)] (edge-clamped), so one
indirect DMA row fetch (per-partition offset) brings all 4 corners of one
sample point.

Default design "g8i6": the quad table is stored in fp8-e3m4 (halves gather
bytes; 4-bit mantissa keeps the deterministic L2 rel err at 1.29e-2, under
the 2e-2 gate — e4m3 measures 2.56e-2 and fails). Per chunk of 128 slots,
4 indirect DMAs (one per 2x2 pool member, 128 x 2KB rows each) fill an fp8
tile; the 16 weighted corner adds run on the TensorEngine as diag-matmuls
(DVE builds diag(w) fp16 from an identity via tensor_scalar at 4x rate; PE
contracts diag @ corner-block into fp32 PSUM, mixed fp16 x fp8 operands).
Max-pool obeys the one-PSUM-operand rule: ScalarE evacuates samples 1/3 to
SBUF fp16, DVE does the three maxes, one DMA stores each chunk. A 6-deep
gather pipeline (gpool bufs) keeps the indirect-DMA queue full — measured
HW is descriptor-rate-bound (~13 ns/descriptor), so gather depth, not
bandwidth, dominates. Measured ~131 us vs the 219 us fp16/DVE baseline.

Older designs kept for comparison: q16 (fp16 + DVE/ScalarE weighted sums),
q16pe, g8 (dma_gather variant — slower: Q7 SWDGE gather underperforms
SWDGE indirect DMA per descriptor), x32 (fp32 fallback).
"""

import numpy as np

POOL = 7
PRE = POOL * 2          # 14
STRIDE = 16.0
C, H, W = 512, 50, 75
N = 512
NCORES = 8
N_LOC = N // NCORES     # 64 ROIs per core
SLOTS = N_LOC * POOL * POOL          # 3136 pooled outputs per core
CHUNKS = (SLOTS + 127) // 128        # 25
SLOT_PAD = CHUNKS * 128              # 3200
NW = 16                              # weights per slot

DESIGN = "g8i6"  # fp8e3 quads via indirect DMA + PE diag-matmul + PSUM pooling

_CACHE = {}
G8_SORT = True   # sort slots by cell for gather DRAM locality


def _axis_corners(s, t, size):
    """Sample positions v -> floor corner v0 and corner weights w0/w1 (fp32)."""
    f32 = np.float32
    base = np.linspace(-1.0, 1.0, PRE, dtype=f32)
    g = s[:, None] * base[None, :] + t[:, None]          # [N, 14]
    v = (g + f32(1.0)) * f32(0.5) * f32(size - 1)
    v0 = np.floor(v)
    w1 = v - v0
    w0 = f32(1.0) - w1
    return v0, w0, w1


def _roi_params(rois):
    f32 = np.float32
    r = rois.astype(f32)
    x1 = r[:, 1] / f32(STRIDE)
    y1 = r[:, 2] / f32(STRIDE)
    x2 = r[:, 3] / f32(STRIDE)
    y2 = r[:, 4] / f32(STRIDE)
    sx = (x2 - x1) / f32(W - 1)
    tx = (x1 + x2 - W + 1) / f32(W - 1)
    sy = (y2 - y1) / f32(H - 1)
    ty = (y1 + y2 - H + 1) / f32(H - 1)
    return sx, tx, sy, ty


def _clip_remap(v0, w0, w1, size, start_max):
    """Clip unit start to [0, start_max]; distribute corner weights onto the
    unit-local positions d = (v0 + c) - start, dropping invalid corners."""
    f32 = np.float32
    start = np.clip(v0, 0, start_max).astype(np.int32)
    wd = np.zeros(v0.shape + (2,), f32)
    for c in range(2):
        vc = v0 + f32(c)
        valid = (vc >= 0) & (vc <= size - 1)
        wc = (w0 if c == 0 else w1) * valid.astype(f32)
        d = vc.astype(np.int64) - start
        for dd in range(2):
            wd[..., dd] += np.where((d == dd) & valid, wc, 0.0).astype(f32)
    return start, wd


def _host_prep_q16(bottom, rois):
    """Quad-table design: featQ fp16 [H*W, 4C]; 4 gathers per chunk."""
    f = bottom[0].transpose(1, 2, 0)                   # [H, W, C] fp32
    fq = np.empty((H, W, 4, C), np.float16)
    fx = f[:, list(range(1, W)) + [W - 1], :]          # x+1 clamped
    fy = f[list(range(1, H)) + [H - 1], :, :]          # y+1 clamped
    fxy = fy[:, list(range(1, W)) + [W - 1], :]
    fq[:, :, 0] = f
    fq[:, :, 1] = fx
    fq[:, :, 2] = fy
    fq[:, :, 3] = fxy
    featQ = np.ascontiguousarray(fq.reshape(H * W, 4 * C))

    sx, tx, sy, ty = _roi_params(rois)
    y0, wy0, wy1 = _axis_corners(sy, ty, H)
    x0, wx0, wx1 = _axis_corners(sx, tx, W)
    ys, wyd = _clip_remap(y0, wy0, wy1, H, H - 1)      # [N,14], [N,14,2]
    xs, wxd = _clip_remap(x0, wx0, wx1, W, W - 1)

    in_maps = []
    for k in range(NCORES):
        sl = slice(k * N_LOC, (k + 1) * N_LOC)
        ys_v = ys[sl].reshape(N_LOC, POOL, 2)          # [n, I, a]
        wy_v = wyd[sl].reshape(N_LOC, POOL, 2, 2)      # [n, I, a, dy]
        xs_v = xs[sl].reshape(N_LOC, POOL, 2)          # [n, J, b]
        wx_v = wxd[sl].reshape(N_LOC, POOL, 2, 2)      # [n, J, b, dx]

        # unit (a, b): row = ys*W + xs -> [n, I, J, a, b]
        idx_all = (
            ys_v[:, :, None, :, None] * W + xs_v[:, None, :, None, :]
        )
        # weight (a, b, dy, dx) -> [n, I, J, a, b, dy, dx]
        w_all = (
            wy_v[:, :, None, :, None, :, None]
            * wx_v[:, None, :, None, :, None, :]
        )
        idx_flat = idx_all.reshape(SLOTS, 4)
        w_flat = w_all.reshape(SLOTS, NW).astype(np.float32)
        idx_pad = np.zeros((SLOT_PAD, 4), np.int32)
        w_pad = np.zeros((SLOT_PAD, NW), np.float32)
        idx_pad[:SLOTS] = idx_flat
        w_pad[:SLOTS] = w_flat

        idx_dev = (
            idx_pad.reshape(CHUNKS, 128, 4)
            .transpose(1, 0, 2).reshape(128, CHUNKS * 4).copy()
        )
        w_dev = (
            w_pad.reshape(CHUNKS, 128, NW)
            .transpose(1, 0, 2).reshape(128, CHUNKS * NW).copy()
        )
        in_maps.append({"featQ": featQ, "idxs": idx_dev, "wts": w_dev,
                        "ident": np.eye(128, dtype=np.float16)})
    return in_maps


def _build_q16(repeat=1):
    import concourse.bacc as bacc
    import concourse.bass as bass
    import concourse.tile as tile
    from concourse import mybir
    from concourse.bass_interp import get_hw_module

    f16 = mybir.dt.float16
    nc = bacc.Bacc("TRN2", target_bir_lowering=False, debug=False,
                   num_devices=NCORES)
    featQ = nc.dram_tensor("featQ", (H * W, 4 * C), f16, kind="ExternalInput")
    idx_d = nc.dram_tensor("idxs", (128, CHUNKS * 4), mybir.dt.int32,
                           kind="ExternalInput")
    wts_d = nc.dram_tensor("wts", (128, CHUNKS * NW), mybir.dt.float32,
                           kind="ExternalInput")
    out_d = nc.dram_tensor("out", (CHUNKS, 128, C), f16,
                           kind="ExternalOutput")

    U = 4 * C  # elements per gathered unit (4 corners)

    with tile.TileContext(nc) as tc:
        with tc.tile_pool(name="cpool", bufs=1) as cpool, \
             tc.tile_pool(name="gpool", bufs=8) as gpool, \
             tc.tile_pool(name="tpool", bufs=6) as tpool, \
             tc.tile_pool(name="opool", bufs=3) as opool:
            idx_sb = cpool.tile([128, CHUNKS * 4], mybir.dt.int32, tag="idx")
            wts_sb = cpool.tile([128, CHUNKS * NW], mybir.dt.float32,
                                tag="wts")
            nc.sync.dma_start(out=idx_sb[:], in_=idx_d[:])
            nc.sync.dma_start(out=wts_sb[:], in_=wts_d[:])

            def body():
                for ch in range(CHUNKS):
                    g = gpool.tile([128, 4 * U], f16, tag="g")
                    for m in range(4):
                        nc.gpsimd.indirect_dma_start(
                            out=g[:, m * U:(m + 1) * U],
                            out_offset=None,
                            in_=featQ[:],
                            in_offset=bass.IndirectOffsetOnAxis(
                                ap=idx_sb[:, ch * 4 + m: ch * 4 + m + 1],
                                axis=0,
                            ),
                        )
                    accs = []
                    for m in range(4):
                        acc = tpool.tile([128, C], f16, tag=f"acc{m}")
                        s1 = tpool.tile([128, C], f16, tag="s1")
                        s2 = tpool.tile([128, C], f16, tag="s2")
                        s3 = tpool.tile([128, C], f16, tag="s3")
                        for q, t in enumerate((acc, s1, s2, s3)):
                            wcol = ch * NW + m * 4 + q
                            src = g[:, m * U + q * C: m * U + (q + 1) * C]
                            wap = wts_sb[:, wcol:wcol + 1]
                            if q < 2:
                                nc.vector.tensor_scalar_mul(t[:], src, wap)
                            else:
                                nc.scalar.mul(t[:], src, wap)
                        nc.vector.tensor_add(acc[:], acc[:], s1[:])
                        nc.vector.tensor_add(s2[:], s2[:], s3[:])
                        nc.vector.tensor_add(acc[:], acc[:], s2[:])
                        accs.append(acc)
                    nc.vector.tensor_max(accs[0][:], accs[0][:], accs[1][:])
                    nc.vector.tensor_max(accs[2][:], accs[2][:], accs[3][:])
                    ot = opool.tile([128, C], f16, tag="o")
                    nc.vector.tensor_max(ot[:], accs[0][:], accs[2][:])
                    nc.sync.dma_start(out=out_d[ch], in_=ot[:])

            if repeat > 1:
                with tc.For_i(0, repeat, 1):
                    body()
            else:
                body()

    nc.compile()
    nc.m = get_hw_module(nc.m)
    return nc


def _build_q16pe(repeat=1):
    """Like q16, but the 16 weighted-corner multiplies + 12 adds run on the
    TensorEngine as diagonal-matrix matmuls accumulating in PSUM (fp32).
    Each diag is built by one cheap DVE tensor_scalar (identity mask x w).
    ScalarE evacuates PSUM -> SBUF; VectorE does the 3 max-pool ops."""
    import concourse.bacc as bacc
    import concourse.bass as bass
    import concourse.tile as tile
    from concourse import mybir
    from concourse.bass_interp import get_hw_module

    f16 = mybir.dt.float16
    f32 = mybir.dt.float32
    nc = bacc.Bacc("TRN2", target_bir_lowering=False, debug=False,
                   num_devices=NCORES)
    featQ = nc.dram_tensor("featQ", (H * W, 4 * C), f16, kind="ExternalInput")
    idx_d = nc.dram_tensor("idxs", (128, CHUNKS * 4), mybir.dt.int32,
                           kind="ExternalInput")
    wts_d = nc.dram_tensor("wts", (128, CHUNKS * NW), f32,
                           kind="ExternalInput")
    id_d = nc.dram_tensor("ident", (128, 128), f16, kind="ExternalInput")
    out_d = nc.dram_tensor("out", (CHUNKS, 128, C), f16,
                           kind="ExternalOutput")

    U = 4 * C

    with tile.TileContext(nc) as tc:
        with tc.tile_pool(name="cpool", bufs=1) as cpool, \
             tc.tile_pool(name="gpool", bufs=8) as gpool, \
             tc.tile_pool(name="dpool", bufs=8) as dpool, \
             tc.tile_pool(name="tpool", bufs=4) as tpool, \
             tc.tile_pool(name="ppool", bufs=2, space="PSUM") as ppool, \
             tc.tile_pool(name="opool", bufs=3) as opool:
            idx_sb = cpool.tile([128, CHUNKS * 4], mybir.dt.int32, tag="idx")
            wts_sb = cpool.tile([128, CHUNKS * NW], f32, tag="wts")
            id_sb = cpool.tile([128, 128], f16, tag="ident")
            nc.sync.dma_start(out=idx_sb[:], in_=idx_d[:])
            nc.sync.dma_start(out=wts_sb[:], in_=wts_d[:])
            nc.sync.dma_start(out=id_sb[:], in_=id_d[:])

            def body():
                for ch in range(CHUNKS):
                    g = gpool.tile([128, 4 * U], f16, tag="g")
                    for m in range(4):
                        nc.gpsimd.indirect_dma_start(
                            out=g[:, m * U:(m + 1) * U],
                            out_offset=None,
                            in_=featQ[:],
                            in_offset=bass.IndirectOffsetOnAxis(
                                ap=idx_sb[:, ch * 4 + m: ch * 4 + m + 1],
                                axis=0,
                            ),
                        )
                    sms = []
                    for m in range(4):
                        pacc = ppool.tile([128, C], f32, tag=f"p{m}",
                                          space="PSUM")
                        for q in range(4):
                            wcol = ch * NW + m * 4 + q
                            dg = dpool.tile([128, 128], f16, tag="d")
                            nc.vector.tensor_scalar_mul(
                                dg[:], id_sb[:], wts_sb[:, wcol:wcol + 1]
                            )
                            nc.tensor.matmul(
                                pacc[:],
                                lhsT=dg[:],
                                rhs=g[:, m * U + q * C: m * U + (q + 1) * C],
                                start=(q == 0),
                                stop=(q == 3),
                            )
                        sm = tpool.tile([128, C], f16, tag=f"s{m}")
                        nc.scalar.copy(sm[:], pacc[:])
                        sms.append(sm)
                    nc.vector.tensor_max(sms[0][:], sms[0][:], sms[1][:])
                    nc.vector.tensor_max(sms[2][:], sms[2][:], sms[3][:])
                    ot = opool.tile([128, C], f16, tag="o")
                    nc.vector.tensor_max(ot[:], sms[0][:], sms[2][:])
                    nc.sync.dma_start(out=out_d[ch], in_=ot[:])

            if repeat > 1:
                with tc.For_i(0, repeat, 1):
                    body()
            else:
                body()

    nc.compile()
    nc.m = get_hw_module(nc.m)
    return nc


def _host_prep_g8(bottom, rois):
    """fp8-e3m4 quad table + batched dma_gather + PE weighted sum.

    featQ8 [H*W, 4C] fp8e3 (quantization: L2 rel err ~1.3e-2, under the 2e-2
    gate). Per chunk of 128 slots one dma_gather fetches 512 quad rows
    (4 sample points per slot); indices int16 in the SWDGE wrapped layout
    (idx i at partition i%16, column i//16, replicated per 16-partition
    group). Weights fp32 per-partition (consumed as tensor_scalar scalars).
    """
    import ml_dtypes

    f = bottom[0].transpose(1, 2, 0)                   # [H, W, C] fp32
    fq = np.empty((H, W, 4, C), np.float32)
    fx = f[:, list(range(1, W)) + [W - 1], :]
    fy = f[list(range(1, H)) + [H - 1], :, :]
    fxy = fy[:, list(range(1, W)) + [W - 1], :]
    fq[:, :, 0] = f
    fq[:, :, 1] = fx
    fq[:, :, 2] = fy
    fq[:, :, 3] = fxy
    featQ8 = np.ascontiguousarray(
        fq.reshape(H * W, 4 * C).astype(ml_dtypes.float8_e3m4)
    )

    sx, tx, sy, ty = _roi_params(rois)
    y0, wy0, wy1 = _axis_corners(sy, ty, H)
    x0, wx0, wx1 = _axis_corners(sx, tx, W)
    ys, wyd = _clip_remap(y0, wy0, wy1, H, H - 1)
    xs, wxd = _clip_remap(x0, wx0, wx1, W, W - 1)

    ident = np.eye(128, dtype=np.float16)
    in_maps = []
    for k in range(NCORES):
        sl = slice(k * N_LOC, (k + 1) * N_LOC)
        ys_v = ys[sl].reshape(N_LOC, POOL, 2)
        wy_v = wyd[sl].reshape(N_LOC, POOL, 2, 2)
        xs_v = xs[sl].reshape(N_LOC, POOL, 2)
        wx_v = wxd[sl].reshape(N_LOC, POOL, 2, 2)

        idx_all = (
            ys_v[:, :, None, :, None] * W + xs_v[:, None, :, None, :]
        )
        w_all = (
            wy_v[:, :, None, :, None, :, None]
            * wx_v[:, None, :, None, :, None, :]
        )
        idx_flat = idx_all.reshape(SLOTS, 4)
        w_flat = w_all.reshape(SLOTS, NW).astype(np.float32)
        # sort slots by their first sample's cell so each gather's 128
        # descriptors hit ascending HBM addresses (better DRAM locality;
        # ~6% on the gather). _assemble inverts via _G8_PERMS.
        if G8_SORT:
            order = np.argsort(idx_flat[:, 0], kind="stable")
        else:
            order = np.arange(SLOTS)
        idx_flat = idx_flat[order]
        w_flat = w_flat[order]
        _G8_PERMS[k] = order
        idx_pad = np.zeros((SLOT_PAD, 4), np.int64)
        w_pad = np.zeros((SLOT_PAD, NW), np.float32)
        idx_pad[:SLOTS] = idx_flat
        w_pad[:SLOTS] = w_flat

        # dma_gather ordering: gathered row i lands at out[i % 128, i // 128];
        # we want out[p, m] = quad row of (slot p, sample m) -> flat[i] with
        # i = m*128 + p. SWDGE wrapped layout: sb[i%16, i//16], replicated
        # across the eight 16-partition groups.
        idx_cols = []
        for ch in range(CHUNKS):
            flat = idx_pad[ch * 128:(ch + 1) * 128, :].T.reshape(512)
            blk = flat.reshape(32, 16).T            # [16, 32]
            idx_cols.append(np.tile(blk, (8, 1)))   # [128, 32]
        idx_dev = np.concatenate(idx_cols, axis=1).astype(np.int16)

        idx32_dev = (
            idx_pad.reshape(CHUNKS, 128, 4)
            .transpose(1, 0, 2).reshape(128, CHUNKS * 4).astype(np.int32)
        )
        w_dev = (
            w_pad.reshape(CHUNKS, 128, NW)
            .transpose(1, 0, 2).reshape(128, CHUNKS * NW).copy()
        )
        in_maps.append({"featQ8": featQ8, "idxs": idx_dev,
                        "idxs32": idx32_dev, "wts": w_dev, "ident": ident})
    return in_maps


def _build_g8(repeat=1, gather="dg", gbufs=3):
    """fp8e3 quad gather + PE diag-matmul weighted sum (lhsT fp16 diag built
    on DVE, rhs fp8e3, PSUM fp32 accumulate) + PSUM max-pool (Act evacuates
    samples 1/3, DVE maxes). gather: "dg" = one dma_gather per chunk,
    "idma" = four indirect DMAs per chunk (measured faster per descriptor)."""
    import concourse.bacc as bacc
    import concourse.bass as bass
    import concourse.tile as tile
    from concourse import mybir
    from concourse.bass_interp import get_hw_module

    f16 = mybir.dt.float16
    f32 = mybir.dt.float32
    f8 = mybir.dt.float8e3
    i16 = mybir.dt.int16
    nc = bacc.Bacc("TRN2", target_bir_lowering=False, debug=False,
                   num_devices=NCORES)
    featQ8 = nc.dram_tensor("featQ8", (H * W, 4 * C), f8, kind="ExternalInput")
    idx_d = nc.dram_tensor("idxs", (128, CHUNKS * 32), i16,
                           kind="ExternalInput")
    idx32_d = nc.dram_tensor("idxs32", (128, CHUNKS * 4), mybir.dt.int32,
                             kind="ExternalInput")
    wts_d = nc.dram_tensor("wts", (128, CHUNKS * NW), f32,
                           kind="ExternalInput")
    id_d = nc.dram_tensor("ident", (128, 128), f16, kind="ExternalInput")
    out_d = nc.dram_tensor("out", (CHUNKS, 128, C), f16,
                           kind="ExternalOutput")

    U = 4 * C   # fp8 elements per quad row (2048 bytes)

    with tile.TileContext(nc) as tc:
        with tc.tile_pool(name="cpool", bufs=1) as cpool, \
             tc.tile_pool(name="gpool", bufs=gbufs) as gpool, \
             tc.tile_pool(name="dpool", bufs=3) as dpool, \
             tc.tile_pool(name="ppool", bufs=2, space="PSUM") as ppool, \
             tc.tile_pool(name="spool", bufs=3) as spool, \
             tc.tile_pool(name="opool", bufs=3) as opool:
            idx_sb = cpool.tile([128, CHUNKS * 32], i16, tag="idx")
            idx32_sb = cpool.tile([128, CHUNKS * 4], mybir.dt.int32,
                                  tag="idx32")
            wts_sb = cpool.tile([128, CHUNKS * NW], f32, tag="wts")
            id_sb = cpool.tile([128, 128], f16, tag="ident")
            if gather == "dg":
                nc.sync.dma_start(out=idx_sb[:], in_=idx_d[:])
            else:
                nc.sync.dma_start(out=idx32_sb[:], in_=idx32_d[:])
            nc.sync.dma_start(out=wts_sb[:], in_=wts_d[:])
            nc.sync.dma_start(out=id_sb[:], in_=id_d[:])

            def body():
                for ch in range(CHUNKS):
                    g = gpool.tile([128, 4, U], f8, tag="g")
                    if gather == "dg":
                        nc.gpsimd.dma_gather(
                            g[:], featQ8[:], idx_sb[:, ch * 32:(ch + 1) * 32],
                            512, 512, U,
                        )
                    else:
                        for m in range(4):
                            nc.gpsimd.indirect_dma_start(
                                out=g[:, m, :],
                                out_offset=None,
                                in_=featQ8[:],
                                in_offset=bass.IndirectOffsetOnAxis(
                                    ap=idx32_sb[:, ch * 4 + m: ch * 4 + m + 1],
                                    axis=0,
                                ),
                            )
                    paccs = []
                    for m in range(4):
                        pm = ppool.tile([128, C], f32, tag=f"p{m}",
                                        space="PSUM")
                        for q in range(4):
                            col = ch * NW + m * 4 + q
                            dg = dpool.tile([128, 128], f16, tag=f"dg{m}{q}")
                            nc.vector.tensor_scalar_mul(
                                dg[:], id_sb[:], wts_sb[:, col:col + 1]
                            )
                            nc.tensor.matmul(
                                pm[:],
                                lhsT=dg[:],
                                rhs=g[:, m, q * C:(q + 1) * C],
                                start=(q == 0),
                                stop=(q == 3),
                            )
                        paccs.append(pm)
                    # HW rule: a TensorTensor may read at most one PSUM
                    # operand. Act evacuates samples 1/3; DVE maxes each
                    # against the PSUM-resident samples 0/2; gpsimd folds.
                    s1 = spool.tile([128, C], f16, tag="s1")
                    nc.scalar.copy(s1[:], paccs[1][:])
                    s3 = spool.tile([128, C], f16, tag="s3")
                    nc.scalar.copy(s3[:], paccs[3][:])
                    m01 = spool.tile([128, C], f16, tag="m01")
                    nc.vector.tensor_max(m01[:], paccs[0][:], s1[:])
                    m23 = spool.tile([128, C], f16, tag="m23")
                    nc.vector.tensor_max(m23[:], paccs[2][:], s3[:])
                    ot = opool.tile([128, C], f16, tag="o")
                    nc.vector.tensor_max(ot[:], m01[:], m23[:])
                    nc.sync.dma_start(out=out_d[ch], in_=ot[:])

            if repeat > 1:
                with tc.For_i(0, repeat, 1):
                    body()
            else:
                body()

    nc.compile()
    nc.m = get_hw_module(nc.m)
    return nc


def _host_prep_x32(bottom, rois):
    """fp32 fallback: featT [H*W, C] fp32; 8 x-pair gathers per chunk."""
    featT = np.ascontiguousarray(
        bottom[0].transpose(1, 2, 0).reshape(H * W, C), dtype=np.float32
    )
    sx, tx, sy, ty = _roi_params(rois)
    f32 = np.float32
    y0, wy0, wy1 = _axis_corners(sy, ty, H)
    yi = np.zeros(y0.shape + (2,), np.int32)
    wy = np.zeros(y0.shape + (2,), f32)
    for c in range(2):
        yc = y0 + f32(c)
        valid = (yc >= 0) & (yc <= H - 1)
        yi[..., c] = np.clip(yc, 0, H - 1).astype(np.int32)
        wy[..., c] = (wy0 if c == 0 else wy1) * valid.astype(f32)
    x0, wx0, wx1 = _axis_corners(sx, tx, W)
    xs, wxh = _clip_remap(x0, wx0, wx1, W, W - 2)

    in_maps = []
    for k in range(NCORES):
        sl = slice(k * N_LOC, (k + 1) * N_LOC)
        yi_v = yi[sl].reshape(N_LOC, POOL, 2, 2)     # [n, I, a, cy]
        wy_v = wy[sl].reshape(N_LOC, POOL, 2, 2)
        xs_v = xs[sl].reshape(N_LOC, POOL, 2)        # [n, J, b]
        wx_v = wxh[sl].reshape(N_LOC, POOL, 2, 2)    # [n, J, b, h]

        idx_all = (
            yi_v[:, :, None, :, None, :] * W
            + xs_v[:, None, :, None, :, None]
        )                                            # [n, I, J, a, b, cy]
        w_all = (
            wy_v[:, :, None, :, None, :, None]
            * wx_v[:, None, :, None, :, None, :]
        )                                            # [n, I, J, a, b, cy, h]
        idx_flat = idx_all.reshape(SLOTS, 8)
        w_flat = w_all.reshape(SLOTS, NW).astype(np.float32)
        idx_pad = np.zeros((SLOT_PAD, 8), np.int32)
        w_pad = np.zeros((SLOT_PAD, NW), np.float32)
        idx_pad[:SLOTS] = idx_flat
        w_pad[:SLOTS] = w_flat

        idx_dev = (
            idx_pad.reshape(CHUNKS, 128, 8)
            .transpose(1, 0, 2).reshape(128, CHUNKS * 8).copy()
        )
        w_dev = (
            w_pad.reshape(CHUNKS, 128, NW)
            .transpose(1, 0, 2).reshape(128, CHUNKS * NW).copy()
        )
        in_maps.append({"featT": featT, "idxs": idx_dev, "wts": w_dev})
    return in_maps


def _build_x32(repeat=1):
    import concourse.bacc as bacc
    import concourse.bass as bass
    import concourse.tile as tile
    from concourse import mybir
    from concourse.bass_interp import get_hw_module

    f32 = mybir.dt.float32
    nc = bacc.Bacc("TRN2", target_bir_lowering=False, debug=False,
                   num_devices=NCORES)
    featT = nc.dram_tensor("featT", (H * W, C), f32, kind="ExternalInput")
    idx_d = nc.dram_tensor("idxs", (128, CHUNKS * 8), mybir.dt.int32,
                           kind="ExternalInput")
    wts_d = nc.dram_tensor("wts", (128, CHUNKS * NW), f32,
                           kind="ExternalInput")
    out_d = nc.dram_tensor("out", (CHUNKS, 128, C), f32,
                           kind="ExternalOutput")

    U = 2 * C

    with tile.TileContext(nc) as tc:
        with tc.tile_pool(name="cpool", bufs=1) as cpool, \
             tc.tile_pool(name="gpool", bufs=3) as gpool, \
             tc.tile_pool(name="tpool", bufs=3) as tpool, \
             tc.tile_pool(name="opool", bufs=3) as opool:
            idx_sb = cpool.tile([128, CHUNKS * 8], mybir.dt.int32, tag="idx")
            wts_sb = cpool.tile([128, CHUNKS * NW], f32, tag="wts")
            nc.sync.dma_start(out=idx_sb[:], in_=idx_d[:])
            nc.sync.dma_start(out=wts_sb[:], in_=wts_d[:])

            def body():
                for ch in range(CHUNKS):
                    g = gpool.tile([128, 8 * U], f32, tag="g")
                    for u in range(8):
                        nc.gpsimd.indirect_dma_start(
                            out=g[:, u * U:(u + 1) * U],
                            out_offset=None,
                            in_=featT[:],
                            in_offset=bass.IndirectOffsetOnAxis(
                                ap=idx_sb[:, ch * 8 + u: ch * 8 + u + 1],
                                axis=0,
                            ),
                        )
                    accs = []
                    for m in range(4):
                        acc = tpool.tile([128, C], f32, tag=f"acc{m}")
                        s1 = tpool.tile([128, C], f32, tag="s1")
                        s2 = tpool.tile([128, C], f32, tag="s2")
                        s3 = tpool.tile([128, C], f32, tag="s3")
                        for q, t in enumerate((acc, s1, s2, s3)):
                            cy, hh = q // 2, q % 2
                            u = 2 * m + cy
                            wcol = ch * NW + u * 2 + hh
                            nc.scalar.mul(
                                t[:],
                                g[:, u * U + hh * C: u * U + (hh + 1) * C],
                                wts_sb[:, wcol:wcol + 1],
                            )
                        nc.vector.tensor_add(acc[:], acc[:], s1[:])
                        nc.vector.tensor_add(s2[:], s2[:], s3[:])
                        nc.vector.tensor_add(acc[:], acc[:], s2[:])
                        accs.append(acc)
                    nc.vector.tensor_max(accs[0][:], accs[0][:], accs[1][:])
                    nc.vector.tensor_max(accs[2][:], accs[2][:], accs[3][:])
                    ot = opool.tile([128, C], f32, tag="o")
                    nc.vector.tensor_max(ot[:], accs[0][:], accs[2][:])
                    nc.sync.dma_start(out=out_d[ch], in_=ot[:])

            if repeat > 1:
                with tc.For_i(0, repeat, 1):
                    body()
            else:
                body()

    nc.compile()
    nc.m = get_hw_module(nc.m)
    return nc


_DESIGNS = {
    "q16": (_host_prep_q16, _build_q16),
    "q16pe": (_host_prep_q16, _build_q16pe),
    "g8": (_host_prep_g8, _build_g8),
    "g8i": (_host_prep_g8,
            lambda repeat=1: _build_g8(repeat, gather="idma")),
    "g8i6": (_host_prep_g8,
             lambda repeat=1: _build_g8(repeat, gather="idma", gbufs=6)),
    "g8i10": (_host_prep_g8,
              lambda repeat=1: _build_g8(repeat, gather="idma", gbufs=10)),
    "x32": (_host_prep_x32, _build_x32),
}


def _get_program(design, repeat=1):
    key = (design, repeat)
    if key not in _CACHE:
        _CACHE[key] = _DESIGNS[design][1](repeat)
    return _CACHE[key]


_G8_PERMS = {}   # per-core slot sort order (g8 designs); see _host_prep_g8


def _assemble(outs, perms=None):
    """outs: list of per-core [CHUNKS, 128, C] arrays -> [N, C, 7, 7]."""
    full = np.empty((N, C, POOL, POOL), np.float32)
    for k, o in enumerate(outs):
        flat = np.asarray(o, np.float32).reshape(SLOT_PAD, C)[:SLOTS]
        if perms is not None:
            unperm = np.empty_like(flat)
            unperm[perms[k]] = flat
            flat = unperm
        full[k * N_LOC:(k + 1) * N_LOC] = (
            flat.reshape(N_LOC, POOL * POOL, C)
            .transpose(0, 2, 1)
            .reshape(N_LOC, C, POOL, POOL)
        )
    return full


def run_hw(bottom, rois, design=DESIGN, repeat=1, trace=False):
    from concourse import bass_utils

    in_maps = _DESIGNS[design][0](np.asarray(bottom), np.asarray(rois))
    nc = _get_program(design, repeat)
    res = bass_utils.run_bass_kernel_spmd(
        nc, in_maps, core_ids=list(range(NCORES)), trace=trace
    )
    perms = _G8_PERMS if design.startswith("g8") else None
    out = _assemble([r["out"] for r in res.results], perms=perms)
    return out, res


def kernel(bottom, rois):
    out, _ = run_hw(bottom, rois)
    return out

